# revision 53
# baseline (speedup 1.0000x reference)
"""Deformable Conv2d (B=4, Cin=64, Cout=128, H=W=128, K=3) on 8 trn2 cores.

Sharding: data-parallel over (batch, H-half): core s -> image s//2,
rows [64*(s%2), +64). All FLOPs on device:
  - offset/modulator 3x3 convs on PE (pos-major out via x-as-lhsT)
  - bilinear corner weights + gather indices on DVE/ACT
  - 4-corner gather via SWDGE dma_gather (512B/descriptor, bf16),
    split across 4 SWDGE queues per 8-row block so all four Q7 core
    pairs generate descriptors concurrently (desc-gen is the gather
    bottleneck at ~9.5ns/idx on one core pair)
  - idx wrap (pos-partition -> 16-partition-wrapped + 8x replicated)
    via a small p-major DRAM bounce with 144B-granule descriptors
    plus one DVE free-dim reorder (vs. 2-byte-granule scatter DMAs)
  - corner combine: one broadcast tensor_tensor multiply per (tap, xc)
  - corner-sum + transpose via PE transpose into PSUM
  - 576->128 einsum on PE (bf16, f32 PSUM)
Small per-block tensors are packed into a few large SBUF tiles with
bitcast views (tile slots pad to 4KB each).
Host side: input layout prep (padded shards, row-pair-duplicated gather
source, weight reordering, constant tables) and output reassembly.
"""

import numpy as np
import ml_dtypes

import concourse.bass as bass
import concourse.bacc as bacc
import concourse.mybir as mybir
from concourse.tile import TileContext
from concourse import library_config

F32 = mybir.dt.float32
BF16 = mybir.dt.bfloat16
I16 = mybir.dt.int16
I32 = mybir.dt.int32
ALU = mybir.AluOpType
_FLOOR_BIAS = -0.5  # HW float->int cast rounds; sim truncates (use 0.0)
ACTF = mybir.ActivationFunctionType

B, Cin, Cout, H, W = 4, 64, 128, 128, 128
Hs = 64                      # rows per shard
PADY = PADX = 4
Hp, Wp = 73, 136             # X2 padded dims
CONV_H, CONV_W = Hs + 2, W + 2   # 66 x 130 conv input (pad 1)
CONV_FLAT = CONV_H * CONV_W      # 8580
ELEM = 256                   # gather element: (xc2, c64, yc2) bf16 = 512B
HB = 8                       # rows per processing block
NBLK = Hs // HB              # 8 blocks
NPOS = HB * W                # 1024 positions per block
NIDX = 9 * NPOS              # 9216 gather idxs per block
NQ = 4                       # SWDGE queues (Q7 core pairs)
TAPS = 9
KH = TAPS * HB               # 72 (k, h) idx slots per position-partition
# tap groups for the offset/modulator conv
# (pair groups use the +1-shifted second half of the x tile; K=128)
GROUPS = [(0, 1), (3, 4), (6, 7), (2,), (5,), (8,)]


def _conv_off(k, h):
    ky, kx = divmod(k, 3)
    return (h + ky) * CONV_W + kx


def build_nc():
    nc = bacc.Bacc("TRN2", num_swdge_queues=NQ)

    xab = nc.dram_tensor("xab", [128, CONV_FLAT], BF16, kind="ExternalInput")
    # gather source: point rows of 128 (c, yc); an element spans two
    # consecutive points (xc) = 256 values, so rows overlap (elem_step=128)
    x2 = nc.dram_tensor("x2", [Hp * Wp + 1, 128], BF16, kind="ExternalInput")
    ck = nc.dram_tensor("ck", [128, 27], F32, kind="ExternalInput")
    chv = nc.dram_tensor("chv", [128, Hs], F32, kind="ExternalInput")
    pvec = nc.dram_tensor("pvec", [128, 4], F32, kind="ExternalInput")
    wconv = nc.dram_tensor("wconv", [128, 6, 27], BF16, kind="ExternalInput")
    # einsum weights: chunk per tap, rows = (c, yc) (yc-duplicated)
    wmain = nc.dram_tensor("wmain", [128, TAPS, 128], BF16,
                           kind="ExternalInput")
    ident = nc.dram_tensor("ident", [128, 128], BF16, kind="ExternalInput")
    out = nc.dram_tensor("out", [128, Hs * W], F32, kind="ExternalOutput")
    # idx bounce scratch, p-major [p, (k, h)]
    idxs_dram = nc.dram_tensor("idxs_scratch", [2, 128, KH], I16,
                               kind="Internal")

    with TileContext(nc) as tc:
        with tc.tile_pool(name="static", bufs=1) as static, \
             tc.tile_pool(name="flda", bufs=5) as flda, \
             tc.tile_pool(name="fldb", bufs=1) as fldb, \
             tc.tile_pool(name="idxp", bufs=5) as idxp, \
             tc.tile_pool(name="iqp", bufs=2) as iqp, \
             tc.tile_pool(name="vpa", bufs=4) as vpa, \
             tc.tile_pool(name="vpb", bufs=3) as vpb, \
             tc.tile_pool(name="stp", bufs=1) as stp, \
             tc.tile_pool(name="outp", bufs=2) as outp, \
             tc.tile_pool(name="pconv", bufs=2, space="PSUM") as pconv, \
             tc.tile_pool(name="ptac", bufs=2, space="PSUM") as ptac, \
             tc.tile_pool(name="pout", bufs=2, space="PSUM") as pout:

            nc.gpsimd.load_library(library_config.mlp)

            # ---- static tiles (bf16 + f32 consts packed) ----
            t_xab = static.tile([128, CONV_FLAT], BF16)
            xcut = 10 * CONV_W
            nc.sync.dma_start(t_xab[:, 0:xcut], xab[:, 0:xcut])
            nc.sync.dma_start(t_xab[:, xcut:], xab[:, xcut:])
            t_sb = static.tile([128, 1632], BF16)   # wconv|wmain|ident|f32
            v_wconv = t_sb[:, 0:162].rearrange("p (g o) -> p g o", g=6)
            v_wmain = t_sb[:, 162:1314].rearrange("p (k o) -> p k o", k=TAPS)
            v_ident = t_sb[:, 1314:1442]
            nc.sync.dma_start(v_wconv, wconv[:])
            nc.sync.dma_start(v_wmain, wmain[:])
            nc.sync.dma_start(v_ident, ident[:])
            t_sf = t_sb[:, 1442:1632].bitcast(F32)  # ck|chv|pvec+consts
            v_ck = t_sf[:, 0:27]
            v_chv = t_sf[:, 27:91]
            v_pvec = t_sf[:, 91:92]
            v_fbias = t_sf[:, 92:93]                # _FLOOR_BIAS
            v_one = t_sf[:, 93:94]                  # 1.0
            v_wpc = t_sf[:, 94:95]                  # float(Wp)
            nc.sync.dma_start(v_ck, ck[:])
            nc.sync.dma_start(v_chv, chv[:])
            nc.sync.dma_start(t_sf[:, 91:95], pvec[:])

            def bcast(v, n):
                # [128,1] per-partition const -> [128, HB, n] broadcast AP
                return bass.AP(tensor=v.tensor, offset=v.offset,
                               ap=[list(v.ap[0]), [0, HB], [0, n]])

            x2ap = x2[:]
            x2ov = bass.AP(tensor=x2ap.tensor, offset=x2ap.offset,
                           ap=[[128, Hp * Wp], [1, ELEM]])

            # per-block state passed from prep stage to compute stage
            state = {}

            def conv_part(blk):
                """offset/modulator conv for one block (PE + ACT copies)."""
                h0 = blk * HB
                # packed per-block bf16 tile: off | wc | i16
                t_fa = flda.tile([128, 576], BF16, tag="fa")
                v_off = t_fa[:, 0:216].rearrange("p (h k) -> p h k", h=HB)
                v_wc = t_fa[:, 216:504].rearrange(
                    "p (h k x y) -> p h k x y", h=HB, k=TAPS, x=2)
                v_i16 = t_fa[:, 504:576].bitcast(I16).rearrange(
                    "p (k h) -> p k h", k=TAPS)
                state[blk] = {"off": v_off, "wc": v_wc, "i16": v_i16}

                # ---- conv: offsets+modulator, pos-major [128(w), h, 27] ----
                for h in range(HB):
                    hg = h0 + h
                    ps = pconv.tile([128, 27], F32, tag="pc")
                    for j, grp in enumerate(GROUPS):
                        kdim = 64 if len(grp) == 1 else 128
                        o = _conv_off(grp[0], hg)
                        nc.tensor.matmul(
                            ps[:], t_xab[:kdim, o:o + W],
                            v_wconv[:kdim, j, :],
                            start=(j == 0), stop=(j == len(GROUPS) - 1))
                    nc.scalar.copy(v_off[:, h, :], ps[:])

            def fields_idx(blk):
                """bilinear fields + idx wrap for one block (DVE + bounce)."""
                h0 = blk * HB
                v_off = state[blk]["off"]
                v_wc = state[blk]["wc"]
                v_i16 = state[blk]["i16"]
                # packed per-block f32 scratch (Fm/i32/fl share one slot
                # via in-place dtype converts; w-temps reuse 3 slots)
                t_fb = fldb.tile([128, 864], F32, tag="fb")
                v_F = t_fb[:, 0:216].rearrange("p (h k) -> p h k", h=HB)
                v_fl = t_fb[:, 216:360].rearrange("p (h k) -> p h k", h=HB)
                v_fli = t_fb[:, 216:360].bitcast(I32).rearrange(
                    "p (h k) -> p h k", h=HB)
                v_fr = t_fb[:, 360:504].rearrange("p (h k) -> p h k", h=HB)
                v_mask = t_fb[:, 504:576].rearrange("p (h k) -> p h k", h=HB)
                v_t1 = t_fb[:, 576:648].rearrange("p (h k) -> p h k", h=HB)
                v_t2 = t_fb[:, 648:720].rearrange("p (h k) -> p h k", h=HB)
                v_t3 = t_fb[:, 720:792].rearrange("p (h k) -> p h k", h=HB)
                v_lin = t_fb[:, 792:864].rearrange("p (h k) -> p h k", h=HB)

                # ---- fields ----
                # F = off + ck[k] (+ h for y cols, + p for x cols)
                nc.vector.tensor_tensor(
                    v_F, v_off,
                    bass.AP(tensor=v_ck.tensor, offset=v_ck.offset,
                            ap=[list(v_ck.ap[0]), [0, HB], [1, 27]]),
                    ALU.add)
                chs = v_chv[:, h0:h0 + HB]
                nc.vector.tensor_tensor(
                    v_F[:, :, 0:9], v_F[:, :, 0:9],
                    bass.AP(tensor=chs.tensor, offset=chs.offset,
                            ap=[list(chs.ap[0]), [1, HB], [0, 9]]),
                    ALU.add)
                # single-src tensor_scalar DVE ops can engage the 2-port
                # perf mode whose SBUF port contends with GpSimd desc-gen
                # (5-16us stalls); use broadcast tensor_tensor instead.
                nc.vector.tensor_tensor(
                    v_F[:, :, 9:18], v_F[:, :, 9:18], bcast(v_pvec, 9),
                    ALU.add)
                # floor via round(x - 0.5): biased copy then in-place
                # f32 -> i32 -> f32 converts
                nc.vector.tensor_tensor(v_fl, v_F[:, :, 0:18],
                                        bcast(v_fbias, 18), ALU.add)
                nc.vector.tensor_tensor(v_fli, v_fl, bcast(v_one, 18),
                                        ALU.mult)
                nc.vector.tensor_tensor(v_fl, v_fli, bcast(v_one, 18),
                                        ALU.mult)
                nc.vector.tensor_tensor(v_fr, v_F[:, :, 0:18], v_fl,
                                        ALU.subtract)
                ty = v_fr[:, :, 0:9]
                tx = v_fr[:, :, 9:18]
                nc.scalar.activation(v_mask, v_F[:, :, 18:27], ACTF.Sigmoid)
                # corner weights; mask folded in. wc[p,h,k,xc,yc]
                nc.vector.tensor_tensor(v_t1, ty, tx, ALU.mult)      # w11
                nc.vector.tensor_tensor(v_wc[:, :, :, 1, 1], v_t1,
                                        v_mask, ALU.mult)
                nc.vector.tensor_tensor(v_t2, tx, v_t1, ALU.subtract)  # w01
                nc.vector.tensor_tensor(v_wc[:, :, :, 1, 0], v_t2,
                                        v_mask, ALU.mult)
                nc.vector.tensor_tensor(v_t3, ty, v_t1, ALU.subtract)  # w10
                nc.vector.tensor_tensor(v_wc[:, :, :, 0, 1], v_t3,
                                        v_mask, ALU.mult)
                nc.vector.tensor_tensor(v_t3, bcast(v_one, 9), ty,
                                        ALU.subtract)                  # 1-ty
                nc.vector.tensor_tensor(v_t3, v_t3, v_t2, ALU.subtract)  # w00
                nc.vector.tensor_tensor(v_wc[:, :, :, 0, 0], v_t3,
                                        v_mask, ALU.mult)
                # linear idx = y0*Wp + x0 (f32 exact) -> int16, free (k, h)
                nc.vector.tensor_tensor(v_lin, v_fl[:, :, 0:9],
                                        bcast(v_wpc, 9), ALU.mult)
                nc.vector.tensor_tensor(v_lin, v_lin, v_fl[:, :, 9:18],
                                        ALU.add)
                nc.vector.tensor_tensor(
                    v_i16.rearrange("p k h -> p h k"), v_lin,
                    bcast(v_one, 9), ALU.mult)

                # ---- idx wrap: [p, (k,h)] -> [r (x8), (k,h,q)] ----
                # bounce to DRAM p-major (contiguous per partition), read
                # back per 16-partition group with (q,k,h) order (144B
                # granules), then DVE-reorder free dims to (k,h,q).
                base = idxs_dram[blk % 2]
                nc.sync.dma_start(base, v_i16.rearrange("p k h -> p (k h)"))
                t_q = iqp.tile([128, KH * 8], I16, tag="iq")
                for g in range(8):
                    src_view = bass.AP(
                        tensor=base.tensor, offset=base.offset,
                        ap=[[KH, 16], [16 * KH, 8], [1, KH]])
                    nc.sync.dma_start(t_q[16 * g:16 * (g + 1)], src_view)
                t_idxw = idxp.tile([128, KH * 8], I16, tag="ix")
                one_r = bass.AP(tensor=v_one.tensor, offset=v_one.offset,
                                ap=[list(v_one.ap[0]), [0, TAPS], [0, HB],
                                    [0, 8]])
                nc.vector.tensor_tensor(
                    t_idxw[:].rearrange("p (k h q) -> p k h q", h=HB, q=8),
                    t_q[:].rearrange("p (q k h) -> p k h q", q=8, k=TAPS),
                    one_r, ALU.mult)
                state[blk]["idxw"] = t_idxw

            def gathers(blk):
                """issue the 4-queue SWDGE gathers for one block."""
                t_idxw = state[blk]["idxw"]
                # tap-aligned split: A = taps 0-4 (depth 4), B = taps 5-8
                # (depth 3); the A gathers are what the combine waits on
                # first, so they get the deeper queueing
                t_va = vpa.tile([128, 5, HB, ELEM], BF16, tag="va")
                t_vb = vpb.tile([128, 4, HB, ELEM], BF16, tag="vb")
                fa = t_va[:].rearrange("p k h e -> p (k h) e")
                fb = t_vb[:].rearrange("p k h e -> p (k h) e")
                chunks = [(fa, 0, 20, 0), (fa, 20, 20, 20),
                          (fb, 0, 16, 40), (fb, 16, 16, 56)]
                for qi, (fl, d0, ns, g0) in enumerate(chunks):
                    nidx_q = ns * 128
                    nc.gpsimd.dma_gather(
                        fl[:, d0:d0 + ns, :], x2ov,
                        t_idxw[:, g0 * 8:(g0 + ns) * 8],
                        nidx_q, nidx_q, ELEM, elem_step=128,
                        single_packet=False, queue_num=qi)
                state[blk]["va"] = t_va
                state[blk]["vb"] = t_vb

            def combine_transpose(blk):
                """corner combine (DVE) + PE transposes for one block."""
                t_va = state[blk]["va"]
                t_vb = state[blk]["vb"]
                v_wc = state[blk]["wc"]
                t_st = stp.tile([128, TAPS, HB, 128], BF16, tag="st")
                state[blk]["st"] = t_st
                for k in range(TAPS):
                    tv, kk = (t_va, k) if k < 5 else (t_vb, k - 5)
                    # combine: U = V * broadcast(wc), in place.
                    # per (tap, xc): ISA allows only 3 free dims
                    for xc in range(2):
                        vv = tv[:, kk, :, xc * 128:(xc + 1) * 128]
                        wv = bass.AP(
                            tensor=v_wc.tensor,
                            offset=v_wc.offset + k * 4 + xc * 2,
                            ap=[list(v_wc.ap[0]), [TAPS * 4, HB],
                                [0, 64], [1, 2]])
                        nc.vector.tensor_tensor(vv, vv, wv, ALU.mult)
                    # x-corner sum into the x0 half (DVE; PSUM cannot
                    # accumulate bf16 transposes on hw)
                    nc.vector.tensor_tensor(
                        tv[:, kk, :, 0:128], tv[:, kk, :, 0:128],
                        tv[:, kk, :, 128:256], ALU.add)
                    # transpose into psum; rows become (c, yc);
                    # y-corner sum happens in einsum (duplicated W rows)
                    vap = tv[:]
                    pt = ptac.tile([128, HB, 128], BF16, tag="pt")
                    for h in range(HB):
                        uap = bass.AP(
                            tensor=vap.tensor,
                            offset=vap.offset + kk * HB * ELEM + h * ELEM,
                            ap=[list(vap.ap[0]), [1, 128]])
                        nc.tensor.matmul(
                            pt[:, h, :], uap, v_ident,
                            start=True, stop=True, is_transpose=True)
                    nc.scalar.copy(t_st[:, k], pt[:])

            def einsum_out(blk):
                """576->128 einsum + output store for one block."""
                t_st = state.pop(blk)["st"]
                # ---- einsum: out[o, pos] += wmain_k^T @ S_T_k ----
                # tap weights stationary: load each wmain_j once, stream
                # both 512-pos chunks into two PSUM accumulation groups
                pos = [pout.tile([128, 512], F32, tag="po", name=f"po{i}")
                       for i in range(NPOS // 512)]
                for j in range(TAPS):
                    stv = t_st[:, j].rearrange("p h w -> p (h w)")
                    for cc in range(NPOS // 512):
                        nc.tensor.matmul(
                            pos[cc][:], v_wmain[:, j, :],
                            stv[:, cc * 512:(cc + 1) * 512],
                            start=(j == 0), stop=(j == TAPS - 1))
                for cc in range(NPOS // 512):
                    t_out = outp.tile([128, 512], F32, tag="out")
                    nc.scalar.copy(t_out[:], pos[cc][:])
                    base_o = blk * NPOS + cc * 512
                    nc.sync.dma_start(
                        out[:, base_o:base_o + 512], t_out[:])

            # two-deep software pipeline: fields/idx for block b+2 are
            # produced during block b's compute, so gather(b+1) (POOL
            # desc-gen, the serial bottleneck) runs concurrently with
            # combine(b) on the DVE instead of serializing with it.
            conv_part(0)
            fields_idx(0)
            gathers(0)
            conv_part(1)
            fields_idx(1)
            gathers(1)
            for blk in range(NBLK):
                # fields/reorder for blk+2 run in the DVE bubble while
                # this block's gather drain completes, so the next
                # gathers issue before (not after) combine(blk)
                if blk + 2 < NBLK:
                    conv_part(blk + 2)
                    fields_idx(blk + 2)
                    gathers(blk + 2)
                combine_transpose(blk)
                einsum_out(blk)

    nc.finalize()
    return nc


# ---------------- host side ----------------

def prep_core_inputs(x, w_off, b_off, w_mod, b_mod, w_reg, s):
    """Build device input dict for shard s (image s//2, rows 64*(s%2)+)."""
    b, half = divmod(s, 2)
    r0 = half * Hs
    xb = np.asarray(x[b], dtype=np.float32)           # [C, H, W]

    # conv input, channel-major padded [64, 66, 130] bf16; second partition
    # half is the same data shifted by +1 element (for pair tap groups)
    xcm = np.zeros((Cin, CONV_H, CONV_W), np.float32)
    ylo = r0 - 1
    sylo, syhi = max(ylo, 0), min(ylo + CONV_H, H)
    xcm[:, sylo - ylo:syhi - ylo, 1:1 + W] = xb[:, sylo:syhi, :]
    xf = xcm.reshape(Cin, CONV_FLAT).astype(ml_dtypes.bfloat16)
    xab = np.zeros((128, CONV_FLAT), ml_dtypes.bfloat16)
    xab[:Cin] = xf
    xab[Cin:, :-1] = xf[:, 1:]

    # X2 gather source: [Hp, Wp, Cin, 2(yc)] -> flat [Hp*Wp (+1), 128]
    X2 = np.zeros((Hp, Wp, Cin, 2), np.float32)
    for yc in range(2):
        ylo = r0 - PADY + yc
        sylo, syhi = max(ylo, 0), min(ylo + Hp, H)
        X2[sylo - ylo:syhi - ylo, PADX:PADX + W, :, yc] = \
            xb[:, sylo:syhi, :].transpose(1, 2, 0)
    x2 = np.zeros((Hp * Wp + 1, Cin * 2), ml_dtypes.bfloat16)
    x2[:Hp * Wp] = X2.reshape(Hp * Wp, Cin * 2).astype(ml_dtypes.bfloat16)

    # decomposed consts: ck[*, 27] per-tap, chv[*, h]=h, pvec[p,1]=p
    ck = np.zeros((128, 27), np.float32)
    for k in range(TAPS):
        ky, kx = divmod(k, 3)
        ck[:, k] = ky + (PADY - 1) + float(b_off[2 * k])
        ck[:, 9 + k] = kx + (PADX - 1) + float(b_off[2 * k + 1])
        ck[:, 18 + k] = float(b_mod[k])
    chv = np.broadcast_to(np.arange(Hs, dtype=np.float32)[None, :],
                          (128, Hs)).copy()
    pvec = np.zeros((128, 4), np.float32)
    pvec[:, 0] = np.arange(128, dtype=np.float32)
    pvec[:, 1] = _FLOOR_BIAS
    pvec[:, 2] = 1.0
    pvec[:, 3] = float(Wp)

    # conv weights [128, 6, 27]: group rows = [c of k0 | c of k1]
    wconv = np.zeros((128, 6, 27), np.float32)
    for j, grp in enumerate(GROUPS):
        for t, k in enumerate(grp):
            ky, kx = divmod(k, 3)
            for o in range(9):
                wconv[t * 64:(t + 1) * 64, j, o] = w_off[2 * o, :, ky, kx]
                wconv[t * 64:(t + 1) * 64, j, 9 + o] = w_off[2 * o + 1, :, ky, kx]
                wconv[t * 64:(t + 1) * 64, j, 18 + o] = w_mod[o, :, ky, kx]
    wconv = wconv.astype(ml_dtypes.bfloat16)

    # main weights [128, 9, 128]: chunk k rows = (c, yc) duplicated
    wmain = np.zeros((128, TAPS, 128), np.float32)
    for k in range(TAPS):
        ky, kx = divmod(k, 3)
        wt = 2.0 * w_reg[:, :, ky, kx].T        # [c, o]
        wmain[0::2, k, :] = wt
        wmain[1::2, k, :] = wt
    wmain = wmain.astype(ml_dtypes.bfloat16)

    return {"xab": xab, "x2": x2, "ck": ck, "chv": chv, "pvec": pvec,
            "wconv": wconv, "wmain": wmain,
            "ident": np.eye(128, dtype=ml_dtypes.bfloat16)}


_NC_CACHE = {}


def _run(x, w_off, b_off, w_mod, b_mod, w_reg, trace=False, **spmd_kwargs):
    from concourse.bass_utils import run_bass_kernel_spmd
    x = np.asarray(x); w_off = np.asarray(w_off); b_off = np.asarray(b_off)
    w_mod = np.asarray(w_mod); b_mod = np.asarray(b_mod)
    w_reg = np.asarray(w_reg)

    if "nc" not in _NC_CACHE:
        _NC_CACHE["nc"] = build_nc()
    nc = _NC_CACHE["nc"]

    in_maps = [prep_core_inputs(x, w_off, b_off, w_mod, b_mod, w_reg, s)
               for s in range(8)]
    res = run_bass_kernel_spmd(nc, in_maps, core_ids=list(range(8)),
                               trace=trace, **spmd_kwargs)
    results = res.results if hasattr(res, "results") else res

    out = np.zeros((B, Cout, H, W), np.float32)
    for s in range(8):
        b, half = divmod(s, 2)
        r0 = half * Hs
        out[b, :, r0:r0 + Hs, :] = \
            np.asarray(results[s]["out"]).reshape(Cout, Hs, W)
    return out, res


def kernel(x, w_off, b_off, w_mod, b_mod, w_reg):
    out, _ = _run(x, w_off, b_off, w_mod, b_mod, w_reg)
    return out


# revision 55
# speedup vs baseline: 1.1872x; 1.1872x over previous
"""Deformable Conv2d (B=4, Cin=64, Cout=128, H=W=128, K=3) on 8 trn2 cores.

Sharding: data-parallel over (batch, H-half): core s -> image s//2,
rows [64*(s%2), +64). All FLOPs on device:
  - offset/modulator 3x3 convs on PE (pos-major out via x-as-lhsT)
  - bilinear corner weights + gather indices on DVE/ACT
  - 4-corner gather via SWDGE dma_gather (512B/descriptor, bf16),
    split across 4 SWDGE queues per 8-row block so all four Q7 core
    pairs generate descriptors concurrently (desc-gen is the gather
    bottleneck at ~9.5ns/idx on one core pair)
  - idx wrap (pos-partition -> 16-partition-wrapped + 8x replicated)
    via a small p-major DRAM bounce with 144B-granule descriptors
    plus one DVE free-dim reorder (vs. 2-byte-granule scatter DMAs)
  - corner combine: one broadcast tensor_tensor multiply per (tap, xc)
  - corner-sum + transpose via PE transpose into PSUM
  - 576->128 einsum on PE (bf16, f32 PSUM)
Small per-block tensors are packed into a few large SBUF tiles with
bitcast views (tile slots pad to 4KB each).
Host side: input layout prep (padded shards, row-pair-duplicated gather
source, weight reordering, constant tables) and output reassembly.
"""

import numpy as np
import ml_dtypes

import concourse.bass as bass
import concourse.bacc as bacc
import concourse.mybir as mybir
from concourse.tile import TileContext
from concourse import library_config

F32 = mybir.dt.float32
BF16 = mybir.dt.bfloat16
I16 = mybir.dt.int16
I32 = mybir.dt.int32
ALU = mybir.AluOpType
_FLOOR_BIAS = -0.5  # HW float->int cast rounds; sim truncates (use 0.0)
ACTF = mybir.ActivationFunctionType

B, Cin, Cout, H, W = 4, 64, 128, 128, 128
Hs = 64                      # rows per shard
PADY = PADX = 4
Hp, Wp = 73, 136             # X2 padded dims
CONV_H, CONV_W = Hs + 2, W + 2   # 66 x 130 conv input (pad 1)
CONV_FLAT = CONV_H * CONV_W      # 8580
ELEM = 256                   # gather element: (xc2, c64, yc2) bf16 = 512B
HB = 8                       # rows per processing block
NBLK = Hs // HB              # 8 blocks
NPOS = HB * W                # 1024 positions per block
NIDX = 9 * NPOS              # 9216 gather idxs per block
NQ = 4                       # SWDGE queues (Q7 core pairs)
TAPS = 9
KH = TAPS * HB               # 72 (k, h) idx slots per position-partition
# tap groups for the offset/modulator conv
# (pair groups use the +1-shifted second half of the x tile; K=128)
GROUPS = [(0, 1), (3, 4), (6, 7), (2,), (5,), (8,)]


def _conv_off(k, h):
    ky, kx = divmod(k, 3)
    return (h + ky) * CONV_W + kx


def build_nc():
    nc = bacc.Bacc("TRN2", num_swdge_queues=NQ)

    xab = nc.dram_tensor("xab", [128, CONV_FLAT], BF16, kind="ExternalInput")
    # gather source: point rows of 128 (c, yc); an element spans two
    # consecutive points (xc) = 256 values, so rows overlap (elem_step=128)
    x2 = nc.dram_tensor("x2", [Hp * Wp + 1, 128], BF16, kind="ExternalInput")
    ck = nc.dram_tensor("ck", [128, 27], F32, kind="ExternalInput")
    chv = nc.dram_tensor("chv", [128, Hs], F32, kind="ExternalInput")
    pvec = nc.dram_tensor("pvec", [128, 4], F32, kind="ExternalInput")
    wconv = nc.dram_tensor("wconv", [128, 6, 27], BF16, kind="ExternalInput")
    # einsum weights: chunk per tap, rows = (c, yc) (yc-duplicated)
    wmain = nc.dram_tensor("wmain", [128, TAPS, 128], BF16,
                           kind="ExternalInput")
    ident = nc.dram_tensor("ident", [128, 128], BF16, kind="ExternalInput")
    out = nc.dram_tensor("out", [128, Hs * W], F32, kind="ExternalOutput")
    # idx bounce scratch, p-major [p, (k, h)]
    idxs_dram = nc.dram_tensor("idxs_scratch", [2, 128, KH], I16,
                               kind="Internal")

    with TileContext(nc) as tc:
        with tc.tile_pool(name="static", bufs=1) as static, \
             tc.tile_pool(name="flda", bufs=5) as flda, \
             tc.tile_pool(name="fldb", bufs=1) as fldb, \
             tc.tile_pool(name="idxp", bufs=5) as idxp, \
             tc.tile_pool(name="iqp", bufs=2) as iqp, \
             tc.tile_pool(name="vp", bufs=3) as vp, \
             tc.tile_pool(name="stp", bufs=2) as stp, \
             tc.tile_pool(name="outp", bufs=2) as outp, \
             tc.tile_pool(name="pconv", bufs=2, space="PSUM") as pconv, \
             tc.tile_pool(name="ptac", bufs=2, space="PSUM") as ptac, \
             tc.tile_pool(name="pout", bufs=2, space="PSUM") as pout:

            nc.gpsimd.load_library(library_config.mlp)

            # ---- static tiles (bf16 + f32 consts packed) ----
            t_xab = static.tile([128, CONV_FLAT], BF16)
            xcut = 10 * CONV_W
            nc.sync.dma_start(t_xab[:, 0:xcut], xab[:, 0:xcut])
            nc.sync.dma_start(t_xab[:, xcut:], xab[:, xcut:])
            t_sb = static.tile([128, 1632], BF16)   # wconv|wmain|ident|f32
            v_wconv = t_sb[:, 0:162].rearrange("p (g o) -> p g o", g=6)
            v_wmain = t_sb[:, 162:1314].rearrange("p (k o) -> p k o", k=TAPS)
            v_ident = t_sb[:, 1314:1442]
            nc.sync.dma_start(v_wconv, wconv[:])
            nc.sync.dma_start(v_wmain, wmain[:])
            nc.sync.dma_start(v_ident, ident[:])
            t_sf = t_sb[:, 1442:1632].bitcast(F32)  # ck|chv|pvec+consts
            v_ck = t_sf[:, 0:27]
            v_chv = t_sf[:, 27:91]
            v_pvec = t_sf[:, 91:92]
            v_fbias = t_sf[:, 92:93]                # _FLOOR_BIAS
            v_one = t_sf[:, 93:94]                  # 1.0
            v_wpc = t_sf[:, 94:95]                  # float(Wp)
            nc.sync.dma_start(v_ck, ck[:])
            nc.sync.dma_start(v_chv, chv[:])
            nc.sync.dma_start(t_sf[:, 91:95], pvec[:])

            def bcast(v, n):
                # [128,1] per-partition const -> [128, HB, n] broadcast AP
                return bass.AP(tensor=v.tensor, offset=v.offset,
                               ap=[list(v.ap[0]), [0, HB], [0, n]])

            x2ap = x2[:]
            x2ov = bass.AP(tensor=x2ap.tensor, offset=x2ap.offset,
                           ap=[[128, Hp * Wp], [1, ELEM]])

            # per-block state passed from prep stage to compute stage
            state = {}

            def conv_part(blk):
                """offset/modulator conv for one block (PE + ACT copies)."""
                h0 = blk * HB
                # packed per-block bf16 tile: off | wc | i16
                t_fa = flda.tile([128, 576], BF16, tag="fa")
                v_off = t_fa[:, 0:216].rearrange("p (h k) -> p h k", h=HB)
                v_wc = t_fa[:, 216:504].rearrange(
                    "p (h k x y) -> p h k x y", h=HB, k=TAPS, x=2)
                v_i16 = t_fa[:, 504:576].bitcast(I16).rearrange(
                    "p (k h) -> p k h", k=TAPS)
                state[blk] = {"off": v_off, "wc": v_wc, "i16": v_i16}

                # ---- conv: offsets+modulator, pos-major [128(w), h, 27] ----
                for h in range(HB):
                    hg = h0 + h
                    ps = pconv.tile([128, 27], F32, tag="pc")
                    for j, grp in enumerate(GROUPS):
                        kdim = 64 if len(grp) == 1 else 128
                        o = _conv_off(grp[0], hg)
                        nc.tensor.matmul(
                            ps[:], t_xab[:kdim, o:o + W],
                            v_wconv[:kdim, j, :],
                            start=(j == 0), stop=(j == len(GROUPS) - 1))
                    nc.scalar.copy(v_off[:, h, :], ps[:])

            def fields_idx(blk):
                """bilinear fields + idx wrap for one block (DVE + bounce)."""
                h0 = blk * HB
                v_off = state[blk]["off"]
                v_wc = state[blk]["wc"]
                v_i16 = state[blk]["i16"]
                # packed per-block f32 scratch (Fm/i32/fl share one slot
                # via in-place dtype converts; w-temps reuse 3 slots)
                t_fb = fldb.tile([128, 864], F32, tag="fb")
                v_F = t_fb[:, 0:216].rearrange("p (h k) -> p h k", h=HB)
                v_fl = t_fb[:, 216:360].rearrange("p (h k) -> p h k", h=HB)
                v_fli = t_fb[:, 216:360].bitcast(I32).rearrange(
                    "p (h k) -> p h k", h=HB)
                v_fr = t_fb[:, 360:504].rearrange("p (h k) -> p h k", h=HB)
                v_mask = t_fb[:, 504:576].rearrange("p (h k) -> p h k", h=HB)
                v_t1 = t_fb[:, 576:648].rearrange("p (h k) -> p h k", h=HB)
                v_t2 = t_fb[:, 648:720].rearrange("p (h k) -> p h k", h=HB)
                v_t3 = t_fb[:, 720:792].rearrange("p (h k) -> p h k", h=HB)
                v_lin = t_fb[:, 792:864].rearrange("p (h k) -> p h k", h=HB)

                # ---- fields ----
                # F = off + ck[k] (+ h for y cols, + p for x cols)
                nc.vector.tensor_tensor(
                    v_F, v_off,
                    bass.AP(tensor=v_ck.tensor, offset=v_ck.offset,
                            ap=[list(v_ck.ap[0]), [0, HB], [1, 27]]),
                    ALU.add)
                chs = v_chv[:, h0:h0 + HB]
                nc.vector.tensor_tensor(
                    v_F[:, :, 0:9], v_F[:, :, 0:9],
                    bass.AP(tensor=chs.tensor, offset=chs.offset,
                            ap=[list(chs.ap[0]), [1, HB], [0, 9]]),
                    ALU.add)
                # single-src tensor_scalar DVE ops can engage the 2-port
                # perf mode whose SBUF port contends with GpSimd desc-gen
                # (5-16us stalls); use broadcast tensor_tensor instead.
                nc.vector.tensor_tensor(
                    v_F[:, :, 9:18], v_F[:, :, 9:18], bcast(v_pvec, 9),
                    ALU.add)
                # floor via round(x - 0.5): biased copy then in-place
                # f32 -> i32 -> f32 converts
                nc.vector.tensor_tensor(v_fl, v_F[:, :, 0:18],
                                        bcast(v_fbias, 18), ALU.add)
                nc.vector.tensor_tensor(v_fli, v_fl, bcast(v_one, 18),
                                        ALU.mult)
                nc.vector.tensor_tensor(v_fl, v_fli, bcast(v_one, 18),
                                        ALU.mult)
                nc.vector.tensor_tensor(v_fr, v_F[:, :, 0:18], v_fl,
                                        ALU.subtract)
                ty = v_fr[:, :, 0:9]
                tx = v_fr[:, :, 9:18]
                nc.scalar.activation(v_mask, v_F[:, :, 18:27], ACTF.Sigmoid)
                # corner weights; mask folded in. wc[p,h,k,xc,yc]
                nc.vector.tensor_tensor(v_t1, ty, tx, ALU.mult)      # w11
                nc.vector.tensor_tensor(v_wc[:, :, :, 1, 1], v_t1,
                                        v_mask, ALU.mult)
                nc.vector.tensor_tensor(v_t2, tx, v_t1, ALU.subtract)  # w01
                nc.vector.tensor_tensor(v_wc[:, :, :, 1, 0], v_t2,
                                        v_mask, ALU.mult)
                nc.vector.tensor_tensor(v_t3, ty, v_t1, ALU.subtract)  # w10
                nc.vector.tensor_tensor(v_wc[:, :, :, 0, 1], v_t3,
                                        v_mask, ALU.mult)
                nc.vector.tensor_tensor(v_t3, bcast(v_one, 9), ty,
                                        ALU.subtract)                  # 1-ty
                nc.vector.tensor_tensor(v_t3, v_t3, v_t2, ALU.subtract)  # w00
                nc.vector.tensor_tensor(v_wc[:, :, :, 0, 0], v_t3,
                                        v_mask, ALU.mult)
                # linear idx = y0*Wp + x0 (f32 exact) -> int16, free (k, h)
                nc.vector.tensor_tensor(v_lin, v_fl[:, :, 0:9],
                                        bcast(v_wpc, 9), ALU.mult)
                nc.vector.tensor_tensor(v_lin, v_lin, v_fl[:, :, 9:18],
                                        ALU.add)
                nc.vector.tensor_tensor(
                    v_i16.rearrange("p k h -> p h k"), v_lin,
                    bcast(v_one, 9), ALU.mult)

                # ---- idx wrap: [p, (k,h)] -> [r (x8), (k,h,q)] ----
                # bounce to DRAM p-major (contiguous per partition), read
                # back per 16-partition group with (q,k,h) order (144B
                # granules), then DVE-reorder free dims to (k,h,q).
                base = idxs_dram[blk % 2]
                nc.sync.dma_start(base, v_i16.rearrange("p k h -> p (k h)"))
                t_q = iqp.tile([128, KH * 8], I16, tag="iq")
                for g in range(8):
                    src_view = bass.AP(
                        tensor=base.tensor, offset=base.offset,
                        ap=[[KH, 16], [16 * KH, 8], [1, KH]])
                    nc.sync.dma_start(t_q[16 * g:16 * (g + 1)], src_view)
                t_idxw = idxp.tile([128, KH * 8], I16, tag="ix")
                one_r = bass.AP(tensor=v_one.tensor, offset=v_one.offset,
                                ap=[list(v_one.ap[0]), [0, TAPS], [0, HB],
                                    [0, 8]])
                nc.vector.tensor_tensor(
                    t_idxw[:].rearrange("p (k h q) -> p k h q", h=HB, q=8),
                    t_q[:].rearrange("p (q k h) -> p k h q", q=8, k=TAPS),
                    one_r, ALU.mult)
                state[blk]["idxw"] = t_idxw

            def gathers(blk):
                """issue the 4-queue SWDGE gathers for one block."""
                t_idxw = state[blk]["idxw"]
                t_v = vp.tile([128, TAPS, HB, ELEM], BF16, tag="v")
                t_v_flat = t_v[:].rearrange("p k h e -> p (k h) e")
                per_q = KH // NQ                   # 18 dst slots of 128 idx
                for qi in range(NQ):
                    nidx_q = per_q * 128
                    nc.gpsimd.dma_gather(
                        t_v_flat[:, qi * per_q:(qi + 1) * per_q, :], x2ov,
                        t_idxw[:, qi * per_q * 8:(qi + 1) * per_q * 8],
                        nidx_q, nidx_q, ELEM, elem_step=128,
                        single_packet=False, queue_num=qi)
                state[blk]["v"] = t_v

            def combine_transpose(blk):
                """corner combine (DVE) + PE transposes for one block."""
                t_v = state[blk]["v"]
                v_wc = state[blk]["wc"]
                t_st = stp.tile([128, TAPS, HB, 128], BF16, tag="st")
                state[blk]["st"] = t_st
                for k in range(TAPS):
                    # combine: U = V * broadcast(wc), in place.
                    # per (tap, xc): ISA allows only 3 free dims
                    for xc in range(2):
                        vv = t_v[:, k, :, xc * 128:(xc + 1) * 128]
                        wv = bass.AP(
                            tensor=v_wc.tensor,
                            offset=v_wc.offset + k * 4 + xc * 2,
                            ap=[list(v_wc.ap[0]), [TAPS * 4, HB],
                                [0, 64], [1, 2]])
                        nc.vector.tensor_tensor(vv, vv, wv, ALU.mult)
                    # x-corner sum into the x0 half (DVE; PSUM cannot
                    # accumulate bf16 transposes on hw)
                    nc.vector.tensor_tensor(
                        t_v[:, k, :, 0:128], t_v[:, k, :, 0:128],
                        t_v[:, k, :, 128:256], ALU.add)
                    # transpose into psum; rows become (c, yc);
                    # y-corner sum happens in einsum (duplicated W rows)
                    vap = t_v[:]
                    pt = ptac.tile([128, HB, 128], BF16, tag="pt")
                    for h in range(HB):
                        uap = bass.AP(
                            tensor=vap.tensor,
                            offset=vap.offset + k * HB * ELEM + h * ELEM,
                            ap=[list(vap.ap[0]), [1, 128]])
                        nc.tensor.matmul(
                            pt[:, h, :], uap, v_ident,
                            start=True, stop=True, is_transpose=True)
                    nc.scalar.copy(t_st[:, k], pt[:])

            def einsum_out(blk):
                """576->128 einsum + output store for one block."""
                t_st = state.pop(blk)["st"]
                # ---- einsum: out[o, pos] += wmain_k^T @ S_T_k ----
                # tap weights stationary: load each wmain_j once, stream
                # both 512-pos chunks into two PSUM accumulation groups
                pos = [pout.tile([128, 512], F32, tag="po", name=f"po{i}")
                       for i in range(NPOS // 512)]
                for j in range(TAPS):
                    stv = t_st[:, j].rearrange("p h w -> p (h w)")
                    for cc in range(NPOS // 512):
                        nc.tensor.matmul(
                            pos[cc][:], v_wmain[:, j, :],
                            stv[:, cc * 512:(cc + 1) * 512],
                            start=(j == 0), stop=(j == TAPS - 1))
                for cc in range(NPOS // 512):
                    t_out = outp.tile([128, 512], F32, tag="out")
                    nc.scalar.copy(t_out[:], pos[cc][:])
                    base_o = blk * NPOS + cc * 512
                    nc.sync.dma_start(
                        out[:, base_o:base_o + 512], t_out[:])

            # two-deep software pipeline: fields/idx for block b+2 are
            # produced during block b's compute, so gather(b+1) (POOL
            # desc-gen, the serial bottleneck) runs concurrently with
            # combine(b) on the DVE instead of serializing with it.
            conv_part(0)
            fields_idx(0)
            gathers(0)
            conv_part(1)
            fields_idx(1)
            gathers(1)
            for blk in range(NBLK):
                # fields/reorder for blk+2 run in the DVE bubble while
                # this block's gather drain completes, so the next
                # gathers issue before (not after) combine(blk)
                if blk + 2 < NBLK:
                    conv_part(blk + 2)
                    fields_idx(blk + 2)
                    gathers(blk + 2)
                combine_transpose(blk)
                einsum_out(blk)

    nc.finalize()
    return nc


# ---------------- host side ----------------

def prep_core_inputs(x, w_off, b_off, w_mod, b_mod, w_reg, s):
    """Build device input dict for shard s (image s//2, rows 64*(s%2)+)."""
    b, half = divmod(s, 2)
    r0 = half * Hs
    xb = np.asarray(x[b], dtype=np.float32)           # [C, H, W]

    # conv input, channel-major padded [64, 66, 130] bf16; second partition
    # half is the same data shifted by +1 element (for pair tap groups)
    xcm = np.zeros((Cin, CONV_H, CONV_W), np.float32)
    ylo = r0 - 1
    sylo, syhi = max(ylo, 0), min(ylo + CONV_H, H)
    xcm[:, sylo - ylo:syhi - ylo, 1:1 + W] = xb[:, sylo:syhi, :]
    xf = xcm.reshape(Cin, CONV_FLAT).astype(ml_dtypes.bfloat16)
    xab = np.zeros((128, CONV_FLAT), ml_dtypes.bfloat16)
    xab[:Cin] = xf
    xab[Cin:, :-1] = xf[:, 1:]

    # X2 gather source: [Hp, Wp, Cin, 2(yc)] -> flat [Hp*Wp (+1), 128]
    X2 = np.zeros((Hp, Wp, Cin, 2), np.float32)
    for yc in range(2):
        ylo = r0 - PADY + yc
        sylo, syhi = max(ylo, 0), min(ylo + Hp, H)
        X2[sylo - ylo:syhi - ylo, PADX:PADX + W, :, yc] = \
            xb[:, sylo:syhi, :].transpose(1, 2, 0)
    x2 = np.zeros((Hp * Wp + 1, Cin * 2), ml_dtypes.bfloat16)
    x2[:Hp * Wp] = X2.reshape(Hp * Wp, Cin * 2).astype(ml_dtypes.bfloat16)

    # decomposed consts: ck[*, 27] per-tap, chv[*, h]=h, pvec[p,1]=p
    ck = np.zeros((128, 27), np.float32)
    for k in range(TAPS):
        ky, kx = divmod(k, 3)
        ck[:, k] = ky + (PADY - 1) + float(b_off[2 * k])
        ck[:, 9 + k] = kx + (PADX - 1) + float(b_off[2 * k + 1])
        ck[:, 18 + k] = float(b_mod[k])
    chv = np.broadcast_to(np.arange(Hs, dtype=np.float32)[None, :],
                          (128, Hs)).copy()
    pvec = np.zeros((128, 4), np.float32)
    pvec[:, 0] = np.arange(128, dtype=np.float32)
    pvec[:, 1] = _FLOOR_BIAS
    pvec[:, 2] = 1.0
    pvec[:, 3] = float(Wp)

    # conv weights [128, 6, 27]: group rows = [c of k0 | c of k1]
    wconv = np.zeros((128, 6, 27), np.float32)
    for j, grp in enumerate(GROUPS):
        for t, k in enumerate(grp):
            ky, kx = divmod(k, 3)
            for o in range(9):
                wconv[t * 64:(t + 1) * 64, j, o] = w_off[2 * o, :, ky, kx]
                wconv[t * 64:(t + 1) * 64, j, 9 + o] = w_off[2 * o + 1, :, ky, kx]
                wconv[t * 64:(t + 1) * 64, j, 18 + o] = w_mod[o, :, ky, kx]
    wconv = wconv.astype(ml_dtypes.bfloat16)

    # main weights [128, 9, 128]: chunk k rows = (c, yc) duplicated
    wmain = np.zeros((128, TAPS, 128), np.float32)
    for k in range(TAPS):
        ky, kx = divmod(k, 3)
        wt = 2.0 * w_reg[:, :, ky, kx].T        # [c, o]
        wmain[0::2, k, :] = wt
        wmain[1::2, k, :] = wt
    wmain = wmain.astype(ml_dtypes.bfloat16)

    return {"xab": xab, "x2": x2, "ck": ck, "chv": chv, "pvec": pvec,
            "wconv": wconv, "wmain": wmain,
            "ident": np.eye(128, dtype=ml_dtypes.bfloat16)}


_NC_CACHE = {}


def _run(x, w_off, b_off, w_mod, b_mod, w_reg, trace=False, **spmd_kwargs):
    from concourse.bass_utils import run_bass_kernel_spmd
    x = np.asarray(x); w_off = np.asarray(w_off); b_off = np.asarray(b_off)
    w_mod = np.asarray(w_mod); b_mod = np.asarray(b_mod)
    w_reg = np.asarray(w_reg)

    if "nc" not in _NC_CACHE:
        _NC_CACHE["nc"] = build_nc()
    nc = _NC_CACHE["nc"]

    in_maps = [prep_core_inputs(x, w_off, b_off, w_mod, b_mod, w_reg, s)
               for s in range(8)]
    res = run_bass_kernel_spmd(nc, in_maps, core_ids=list(range(8)),
                               trace=trace, **spmd_kwargs)
    results = res.results if hasattr(res, "results") else res

    out = np.zeros((B, Cout, H, W), np.float32)
    for s in range(8):
        b, half = divmod(s, 2)
        r0 = half * Hs
        out[b, :, r0:r0 + Hs, :] = \
            np.asarray(results[s]["out"]).reshape(Cout, Hs, W)
    return out, res


def kernel(x, w_off, b_off, w_mod, b_mod, w_reg):
    out, _ = _run(x, w_off, b_off, w_mod, b_mod, w_reg)
    return out
